# revision 71
# baseline (speedup 1.0000x reference)
"""Causal self-attention (B=4, T=2048, C=768, H=12) on 8 NeuronCores.

Sharding: core <-> (batch b = core//2, heads h0 = 6*(core%2) .. h0+5).
Each core computes its 6 heads' attention plus the partial output projection;
the host sums the two half-head partials per batch.

Device algorithm (per core), v3:
  1. QKV via 3-pass residual-compensated fp8e4 DoubleRow matmuls:
     acc = X8@W'8 + dX8@W'8 + x8@dW'8 with X=16x, W'=32W (Q cols pre-scaled
     by 0.125), evicted with a single 1/512 power-of-2 scale. fp16-class
     accuracy at 1/4 the fp16 PE cost.
  2. Scores S^T[k, q] per head in fp16, diagonal blocks narrowed to minimal
     width and packed shifted so each 2-block PSUM tile exps in ONE ACT call.
  3. exp on ACT -> E fp16 tiles; gpsimd affine_select zeroes the 128-wide
     diagonal windows.
  4. PV in [q, d] orientation: lhsT = E[k, q-slice], rhs = Vaug[k, 65] per
     head (ones column yields the softmax denominator); out [128q, 130] PSUM
     per head-pair. The i-range per 128-query block is exactly causal-minimal.
  5. Normalize on DVE: reciprocal of the two denominator columns + one
     stride-0-broadcast multiply -> OF fp16 [128q, 128hd].
  6. Batched DMA transpose OF[(j,b)] [128q, 384hd] -> OT[j] [384hd, 128q]
     (per-j OT tiles keep proj deps local to a j-group).
  7. Proj in fp16: out[t, C] = sum_p OT[j][:, 512p+...].T @ Wp[p].

Scheduling (j-major single pipeline): groups (p, j) ordered j-major; group
(0, j) weaves this j's QKV m-tiles behind its first score tiles; group (1, j)
weaves this j's V tiles; PV of group g runs one group behind, interleaved
k-tile by k-tile; proj(j) is queued when PV(2, j) finishes and drains in the
shadow of later groups; the final group's PV chases its own scores.
"""

import os
import numpy as np
import ml_dtypes

import concourse.bass as bass
import concourse.mybir as mybir
import concourse.tile as tile
from concourse import bacc
from concourse.bass_utils import run_bass_kernel_spmd

F32 = mybir.dt.float32
F16 = mybir.dt.float16
F8 = mybir.dt.float8e4
E4M3 = ml_dtypes.float8_e4m3fn

T = 2048
C = 768
D = 64
HPC = 6          # heads per core
NT = 16          # T / 128
NJ = 4           # T / 512
EXP = mybir.ActivationFunctionType.Exp
DR = mybir.MatmulPerfMode.DoubleRow
SC = 1.0 / 512.0

EPL_BUFS = int(os.environ.get("KEPL", "34"))


def _off(i, j):
    return max(0, 128 * i - 512 * j)


def _emit(nc, tc, xa, xb, xc, wqk, wv, dwv, wp, out):
    from contextlib import ExitStack
    with ExitStack() as ctx:
        pp = ctx.enter_context(tc.tile_pool(name="persist", bufs=1))

        # persistent SBUF tiles
        qk = [pp.tile([128, T], F16, tag=f"qk{m}", name=f"qk{m}") for m in range(6)]
        vaug = [pp.tile([128, HPC * (D + 1)], F16, tag=f"v{t}", name=f"vaug{t}")
                for t in range(NT)]
        OT = [pp.tile([128, 3 * 512], F16, tag=f"otj{j}", name=f"OTj{j}")
              for j in range(NJ)]
        OF = {(j, b): pp.tile([128, 384], F16, tag=f"of{j}{b}", name=f"OF{j}{b}")
              for j in range(NJ) for b in range(4)}
        wp_t = [pp.tile([128, C], F16, tag=f"wp{p}", name=f"wp{p}") for p in range(3)]
        # x tiles: [128, 3, 2, T] fp8 (c-pair P, pair-half, token), one per
        # version (X8, dX8, x8); col = 4096*P + 2048*half + t
        xt = {v: pp.tile([128, 3 * 2 * T], F8, tag=f"x{v}", name=f"x{v}")
              for v in "abc"}
        # packed QK weights: per c-pair P one [128, 2, 1536] fp8 tile; col =
        # 512*(m%3) + 256*ver + 128*(m//3)  (pair-0 m-tiles first for startup)
        wqkp_t = [pp.tile([128, 2 * 1536], F8, tag=f"wq{P}", name=f"wq{P}")
                  for P in range(3)]
        wv_t = {v: [pp.tile([128, 2 * HPC * D], F8, tag=f"wv{v}{P}", name=f"wv{v}{P}")
                    for P in range(3)] for v in "mr"}

        epl = ctx.enter_context(tc.tile_pool(name="epool", bufs=EPL_BUFS))
        attsm = ctx.enter_context(tc.tile_pool(name="attsm", bufs=1))
        osb = ctx.enter_context(tc.tile_pool(name="outsb", bufs=3))
        sp = ctx.enter_context(tc.tile_pool(name="spsum", bufs=2, space="PSUM"))
        pvp = ctx.enter_context(tc.tile_pool(name="pvpsum", bufs=2, space="PSUM"))
        # shared [128, 512] f32 PSUM pool: QKV m/v tiles AND proj chunks
        qp = ctx.enter_context(tc.tile_pool(name="qp", bufs=2, space="PSUM"))

        warm = attsm.tile([1, 8], F32, tag="warm", name="warmup")
        nc.vector.memset(warm[:], 0.0)
        nc.scalar.activation(warm[0:1, 0:8], warm[0:1, 0:8], EXP)

        # PE warm-up: keep the tensor engine busy through the initial DMA
        # window so the first real matmuls run at full clock (the cost model
        # ramps the PE clock after ~3us of continuous activity).
        NWARM = int(os.environ.get("KWARM", "10"))
        if NWARM:
            wpe = attsm.tile([1, 512], F16, tag="wpe", name="warm_pe")
            nc.vector.memset(wpe[:], 0.0)
            wps = sp.tile([128, 1024], F32, tag="s", name="warm_ps")
            for i in range(NWARM):
                nc.tensor.matmul(out=wps[:, 0:512], lhsT=wpe[0:1, 0:128],
                                 rhs=wpe[0:1, 0:512], start=True, stop=True)

        def x_rhs(v, P, tsl):
            return xt[v].rearrange("p (P two t) -> p P two t", P=3, two=2)[:, P, :, tsl]

        # ---------------- generators --------------------------------------
        def gen_scores(p, j, es):
            """scores + exp + mask for group (p, j). es[(s, k)] = E tile.
            Yields after each k-tile."""
            ni = 4 * j + 4
            QTm, KTm = qk[p], qk[3 + p]
            for k in range(ni // 2):
                i0 = 2 * k
                w0 = 512 - _off(i0, j)
                w1 = 512 - _off(i0 + 1, j)
                for s in (0, 1):
                    ss = sp.tile([128, 1024], F32, tag="s", name=f"s{p}{j}{k}{s}")
                    b0 = 64 * s
                    for idx, (cb, w) in enumerate(((0, w0), (w0, w1))):
                        i = i0 + idx
                        o = _off(i, j)
                        nc.tensor.matmul(
                            out=ss[:, cb:cb + w],
                            lhsT=KTm[b0:b0 + 64, 128 * i:128 * (i + 1)],
                            rhs=QTm[b0:b0 + 64, 512 * j + o:512 * (j + 1)],
                            start=True, stop=True,
                        )
                    e = epl.tile([128, 1024], F16, tag="e", name=f"e{p}{j}{k}{s}")
                    nc.scalar.activation(e[:, 0:w0 + w1], ss[:, 0:w0 + w1], EXP)
                    for idx, cb in ((0, 0), (1, w0)):
                        if i0 + idx >= 4 * j:
                            win = e[:, cb:cb + 128]
                            nc.gpsimd.affine_select(
                                out=win, in_=win,
                                pattern=[[1, 128]],
                                compare_op=mybir.AluOpType.is_ge,
                                fill=0.0, base=0, channel_multiplier=-1,
                            )
                    es[(s, k)] = e
                yield

        def gen_pv(p, j, es):
            """PV + normalize (+ transpose / proj queue for pair 2) for group
            (p, j), per 128-q block. Yields after each q-block."""
            for b in range(4):
                imax = 4 * j + b
                pvt = pvp.tile([128, 130], F32, tag="pv", name=f"pv{p}{j}{b}")
                for i in range(imax + 1):
                    k, odd = divmod(i, 2)
                    cb = 0 if not odd else 512 - _off(2 * k, j)
                    qoff = 128 * b - _off(i, j)
                    for s in (0, 1):
                        nc.tensor.matmul(
                            out=pvt[:, 65 * s:65 * s + 65],
                            lhsT=es[(s, k)][:, cb + qoff:cb + qoff + 128],
                            rhs=vaug[i][:, 65 * (2 * p + s):65 * (2 * p + s) + 65],
                            start=(i == 0 and s == 0),
                            stop=(i == imax and s == 1),
                        )
                r = attsm.tile([128, 2], F32, tag="r", bufs=2, name=f"r{p}{j}{b}")
                pv3 = pvt.rearrange("q (g c) -> q g c", c=65)
                r3 = r.rearrange("q (g o) -> q g o", o=1)
                nc.vector.reciprocal(r3, pv3[:, :, 64:65])
                of3 = OF[(j, b)][:, 128 * p:128 * (p + 1)] \
                    .rearrange("q (g c) -> q g c", c=64)
                r_bc = r.rearrange("q (g o) -> q g o", o=1).broadcast_to([128, 2, 64])
                nc.vector.tensor_mul(of3, pv3[:, :, 0:64], r_bc)
                if p == 2:
                    # all 3 pairs of OF[(j, b)] done: one batched transpose
                    # [128 q, 384 hd] -> logical [384 hd, 128 q] into OT[j]
                    nc.sync.dma_start_transpose(
                        out=OT[j].rearrange("d (g t) -> d g t", g=3)
                            [:, :, 128 * b:128 * (b + 1)],
                        in_=OF[(j, b)][:])
                yield
            if p == 2:
                pending_proj.append(j)

        pending_proj = []

        def emit_proj_t(t, act_evict=False, alt_pool=False):
            """projection for one 128-row t-block; yields between chunks.
            act_evict: evacuate the 256-col chunk on the (tail-idle) ACT
            engine; alt_pool: draw PSUM from the (tail-idle) scores pool so
            chunk matmuls stop waiting on evictions."""
            jt, bt = divmod(t, 4)
            ob = osb.tile([128, C], F32, tag="ob", name=f"ob{t}")
            for eo, el in ((0, 512), (512, 256)):
                if alt_pool:
                    ps = sp.tile([128, 1024], F32, tag="s", name=f"pj{t}_{eo}")
                else:
                    ps = qp.tile([128, 512], F32, tag="qkv", name=f"pj{t}_{eo}")
                for p in range(3):
                    nc.tensor.matmul(
                        out=ps[:, 0:el],
                        lhsT=OT[jt][:, 512 * p + 128 * bt:512 * p + 128 * (bt + 1)],
                        rhs=wp_t[p][:, eo:eo + el],
                        start=(p == 0), stop=(p == 2),
                    )
                if act_evict and el == 256:
                    nc.scalar.activation(ob[:, eo:eo + el], ps[:, 0:el],
                                         mybir.ActivationFunctionType.Copy)
                else:
                    nc.vector.tensor_copy(ob[:, eo:eo + el], ps[:, 0:el])
                yield
            nc.sync.dma_start(out=out[128 * t:128 * (t + 1), :], in_=ob[:])

        def gen_proj(j):
            tail = j == NJ - 1   # runs after the last exp; ACT + S-pool idle
            for t in range(4 * j, 4 * j + 4):
                yield from emit_proj_t(t, act_evict=tail, alt_pool=tail)

        def drain(g):
            for _ in g:
                pass

        def step(g):
            return next(g, "done") != "done"

        # ---------------- loads -------------------------------------------
        xsrc = {"a": xa, "b": xb, "c": xc}

        def dma_x_chunk(t0, t1):
            # one 3D DMA per (version, pair-half): all 3 c-pairs of a t-range
            for v in "abc":
                for par in range(2):
                    nc.sync.dma_start(
                        out=xt[v].rearrange("p (P rest) -> p P rest", P=3)
                            [:, :, 2048 * par + t0:2048 * par + t1],
                        in_=xsrc[v].rearrange("(P q) t -> q P t", P=3)
                            [128 * par:128 * (par + 1), :, t0:t1])

        for P in range(3):   # wave 1: pair-0 m-tile weights (main + resid)
            nc.sync.dma_start(
                out=wqkp_t[P].rearrange("p (two m) -> p two m", two=2)[:, :, 0:512],
                in_=wqk[256 * P:256 * (P + 1), 0:512]
                    .rearrange("(two p) m -> p two m", two=2))
        dma_x_chunk(0, 512)
        for P in range(3):   # wave 2: remaining QK weight columns
            nc.sync.dma_start(
                out=wqkp_t[P].rearrange("p (two m) -> p two m", two=2)
                    [:, :, 512:1536],
                in_=wqk[256 * P:256 * (P + 1), 512:1536]
                    .rearrange("(two p) m -> p two m", two=2))
        dma_x_chunk(512, 1024)
        for v, src in (("m", wv), ("r", dwv)):
            for P in range(3):
                nc.sync.dma_start(
                    out=wv_t[v][P].rearrange("p (two m) -> p two m", two=2),
                    in_=src[256 * P:256 * (P + 1), :]
                        .rearrange("(two p) m -> p two m", two=2))
        for p in range(3):
            nc.sync.dma_start(out=wp_t[p][:], in_=wp[128 * p:128 * (p + 1), :])

        PASSES = [("a", "m"), ("b", "m"), ("c", "r")]

        def emit_qk(m, j, act_evict=False):
            ps = qp.tile([128, 512], F32, tag="qkv", name=f"qkps{m}_{j}")
            n = 0
            for xv, wver in PASSES:
                col = 512 * (m % 3) + 256 * (wver == "r") + 128 * (m // 3)
                for P in range(3):
                    nc.tensor.matmul(
                        out=ps[:],
                        lhsT=wqkp_t[P].rearrange("p (two m) -> p two m", two=2)
                             [:, :, col:col + 128],
                        rhs=x_rhs(xv, P, slice(512 * j, 512 * (j + 1))),
                        start=(n == 0), stop=(n == 8),
                        perf_mode=DR,
                    )
                    n += 1
            dst = qk[m][:, 512 * j:512 * (j + 1)]
            if act_evict:
                nc.scalar.activation(dst, ps[:],
                                     mybir.ActivationFunctionType.Copy, scale=SC)
            else:
                nc.vector.tensor_scalar_mul(dst, ps[:], SC)

        def emit_v(t):
            nc.vector.memset(vaug[t][:], 1.0)
            ps = qp.tile([128, 512], F32, tag="qkv", name=f"vps{t}")[:, 0:HPC * D]
            n = 0
            for xv, wver in PASSES:
                for P in range(3):
                    nc.tensor.matmul(
                        out=ps[:],
                        lhsT=x_rhs(xv, P, slice(128 * t, 128 * (t + 1))),
                        rhs=wv_t[wver][P].rearrange("p (two m) -> p two m", two=2),
                        start=(n == 0), stop=(n == 8),
                        perf_mode=DR,
                    )
                    n += 1
            dst = vaug[t].rearrange("p (h c) -> p h c", c=D + 1)[:, :, 0:D]
            src = ps.rearrange("p (h c) -> p h c", c=D)
            nc.vector.tensor_scalar_mul(dst, src, SC)

        # ---------------- unified j-major pipeline ------------------------
        emit_qk(0, 0)
        emit_qk(3, 0, act_evict=True)   # parallel first evictions (ACT idle)
        prev = None
        proj_g = None
        proj_g2 = None
        groups = [(p, j) for j in range(NJ) for p in range(3)]
        for gi, (p, j) in enumerate(groups):
            es = {}
            gs = gen_scores(p, j, es)
            gpv = gen_pv(*prev) if prev is not None else None
            last = gi == len(groups) - 1
            self_pv = gen_pv(p, j, es) if last else None
            # per-group woven side work (popped one item per driver step)
            side = []
            if p == 0:
                side += [lambda jj=j: (emit_qk(1, jj), emit_qk(4, jj)),
                         lambda jj=j: (emit_qk(2, jj), emit_qk(5, jj))]
                if j == 0:
                    side.append(lambda: dma_x_chunk(1024, 2048))
            elif p == 1:
                side += [lambda tt=tt: emit_v(tt)
                         for tt in range(4 * j, 4 * j + 4)]
            elif j + 1 < NJ:
                # prefetch next j's first m-tiles behind pair-2's scores
                side.append(lambda jj=j + 1: (emit_qk(0, jj), emit_qk(3, jj)))
            ks = 0
            gs_alive = True
            alive = True
            while alive:
                alive = False
                if gs_alive:
                    gs_alive = step(gs)
                    if gs_alive:
                        ks += 1
                        alive = True
                if side:
                    side.pop(0)()
                    alive = True
                if gpv is not None and step(gpv):
                    alive = True
                if proj_g is not None and not step(proj_g):
                    proj_g = None
                if proj_g2 is not None and not step(proj_g2):
                    proj_g2 = None
                if proj_g is None and pending_proj:
                    proj_g = gen_proj(pending_proj.pop(0))
                elif (proj_g2 is None and pending_proj
                      and pending_proj[0] == NJ - 1):
                    # final j: distinct PSUM pool, can run beside proj_g
                    proj_g2 = gen_proj(pending_proj.pop(0))
                if self_pv is not None and ks >= 2 * j + 1:
                    if step(self_pv):
                        alive = True
            prev = (p, j, es) if self_pv is None else None
        # tail: remaining PV + projections
        if prev is not None:
            gpv = gen_pv(*prev)
            while step(gpv):
                if proj_g is not None and not step(proj_g):
                    proj_g = None
                if proj_g2 is not None and not step(proj_g2):
                    proj_g2 = None
                if proj_g is None and pending_proj:
                    proj_g = gen_proj(pending_proj.pop(0))
                elif (proj_g2 is None and pending_proj
                      and pending_proj[0] == NJ - 1):
                    # final j: distinct PSUM pool, can run beside proj_g
                    proj_g2 = gen_proj(pending_proj.pop(0))
        gens = [g for g in (proj_g, proj_g2) if g is not None]
        while pending_proj:
            gens.append(gen_proj(pending_proj.pop(0)))
        while gens:   # round-robin the remaining proj groups (distinct pools)
            gens = [g for g in gens if step(g)]


_NC_CACHE = None


def build_nc():
    global _NC_CACHE
    if _NC_CACHE is not None:
        return _NC_CACHE
    nc = bacc.Bacc(trn_type="TRN2")
    xa = nc.dram_tensor("xa", [C, T], F8, kind="ExternalInput").ap()
    xb = nc.dram_tensor("xb", [C, T], F8, kind="ExternalInput").ap()
    xc = nc.dram_tensor("xc", [C, T], F8, kind="ExternalInput").ap()
    wqk = nc.dram_tensor("wqk", [C, 2 * C], F8, kind="ExternalInput").ap()
    wv = nc.dram_tensor("wv", [C, HPC * D], F8, kind="ExternalInput").ap()
    dwv = nc.dram_tensor("dwv", [C, HPC * D], F8, kind="ExternalInput").ap()
    wp = nc.dram_tensor("wp", [HPC * D, C], F16, kind="ExternalInput").ap()
    out = nc.dram_tensor("out", [T, C], F32, kind="ExternalOutput").ap()
    with tile.TileContext(nc) as tc:
        _emit(nc, tc, xa, xb, xc, wqk, wv, dwv, wp, out)
    nc.compile()
    _NC_CACHE = nc
    return nc


def make_in_maps(x, W_attn, W_proj):
    x = np.asarray(x, dtype=np.float32)
    W_attn = np.asarray(W_attn, dtype=np.float32)
    W_proj = np.asarray(W_proj, dtype=np.float32)

    def q8(a):
        return a.astype(E4M3)

    in_maps = []
    for core in range(8):
        b = core // 2
        h0 = HPC * (core % 2)
        xt = np.ascontiguousarray(x[b].T)                  # [C, T]
        X = 16.0 * xt
        X8 = q8(X)
        dX8 = q8(X - X8.astype(np.float32))
        x8 = q8(xt)
        q_cols = W_attn[:, 64 * h0:64 * h0 + 384] * np.float32(0.125)
        k_cols = W_attn[:, 768 + 64 * h0:768 + 64 * h0 + 384]
        Wqk = np.concatenate([q_cols, k_cols], axis=1) * np.float32(32.0)
        Wqk8 = q8(Wqk)
        dWqk8 = q8(16.0 * (Wqk - Wqk8.astype(np.float32)))
        # pack [main | resid] per m-pair: col = 512*(m%3) + 256*ver + 128*(m//3)
        Wpk = np.zeros((768, 1536), dtype=E4M3)
        for pg, (ma, mb) in enumerate([(0, 3), (1, 4), (2, 5)]):
            Wpk[:, 512 * pg:512 * pg + 128] = Wqk8[:, 128 * ma:128 * (ma + 1)]
            Wpk[:, 512 * pg + 128:512 * pg + 256] = Wqk8[:, 128 * mb:128 * (mb + 1)]
            Wpk[:, 512 * pg + 256:512 * pg + 384] = dWqk8[:, 128 * ma:128 * (ma + 1)]
            Wpk[:, 512 * pg + 384:512 * pg + 512] = dWqk8[:, 128 * mb:128 * (mb + 1)]
        Wv = W_attn[:, 1536 + 64 * h0:1536 + 64 * h0 + 384] * np.float32(32.0)
        Wv8 = q8(Wv)
        dWv8 = q8(16.0 * (Wv - Wv8.astype(np.float32)))
        wp = np.ascontiguousarray(W_proj[64 * h0:64 * h0 + 384, :]).astype(np.float16)
        in_maps.append({
            "xa": X8, "xb": dX8, "xc": x8,
            "wqk": Wpk,
            "wv": np.ascontiguousarray(Wv8),
            "dwv": np.ascontiguousarray(dWv8),
            "wp": wp,
        })
    return in_maps


def kernel(x, W_attn, W_proj, _trace=False, _trace_kwargs=None):
    nc = build_nc()
    in_maps = make_in_maps(x, W_attn, W_proj)
    res = run_bass_kernel_spmd(nc, in_maps, list(range(8)), trace=_trace,
                               **(_trace_kwargs or {}))
    outs = [res.results[c]["out"] for c in range(8)]
    y = np.stack([outs[2 * b] + outs[2 * b + 1] for b in range(4)]).astype(np.float32)
    if _trace:
        return y, res
    return y
